# revision 1
# baseline (speedup 1.0000x reference)
"""Trainium2 Bass kernel for per-token multi-head self-attention.

Computation (per token t):
  q,k,v = x @ W{q,k,v}.T ; scores = (q_t k_t^T)/sqrt(128) over heads [16x16]
  out_t = softmax(scores) @ v_t ; y = out @ Wo.T

Sharding: data-parallel over the 16384 tokens -> 8 cores x 2048 tokens.
All activations flow on-chip in transposed ([feature, token]) layout; the
host pre-transposes x shards and weights so every matmul operand loads
naturally with the contraction dim on partitions (no on-chip transposes for
the 4 big matmuls). fp32r (full-rate tf32-like) for the big matmuls.

Middle stage per 4-token group: per-token 16x16 score matmuls -> exp (ACT)
into a block-diagonal [128,64] attn matrix -> one AV matmul against the
PE-transposed [4tok x 32, d] V block (with a ones column producing the
softmax normalizer Z) -> per-partition 1/Z scale -> PE-transpose back.
"""
import math
from contextlib import ExitStack

import numpy as np

NCORES = 8
E = 2048          # hidden
NH = 16           # heads
HD = 128          # head dim
TPC = 2048        # tokens per core
TC = 512          # token chunk in pass B
P = 128

_cached = {}


def _build_program():
    import concourse.bass as bass
    import concourse.tile as tile
    from concourse import bacc, mybir
    from concourse.masks import make_identity

    f32 = mybir.dt.float32
    f32r = mybir.dt.float32r

    nc = bacc.Bacc("TRN2", target_bir_lowering=False, debug=False)

    xT_d = nc.dram_tensor("xT", [E, TPC], f32r, kind="ExternalInput").ap()
    WqT_d = nc.dram_tensor("WqT", [E, E], f32r, kind="ExternalInput").ap()
    WkT_d = nc.dram_tensor("WkT", [E, E], f32r, kind="ExternalInput").ap()
    WvT_d = nc.dram_tensor("WvT", [E, E], f32r, kind="ExternalInput").ap()
    WoT_d = nc.dram_tensor("WoT", [E, E], f32r, kind="ExternalInput").ap()
    yT_d = nc.dram_tensor("yT", [E, TPC], f32, kind="ExternalOutput").ap()

    qT_d = nc.dram_tensor("qT_scr", [E, TPC], f32).ap()
    kT_d = nc.dram_tensor("kT_scr", [E, TPC], f32).ap()
    vT_d = nc.dram_tensor("vT_scr", [E, TPC], f32).ap()

    NE = E // P   # 16 k-tiles
    NO = E // P   # 16 o-tiles
    SC = 1.0 / math.sqrt(HD)

    with tile.TileContext(nc) as tc, ExitStack() as ctx:
        glob = ctx.enter_context(tc.tile_pool(name="glob", bufs=1))
        ident = glob.tile([P, P], f32)
        make_identity(nc, ident)

        # ============ PASS A: qT/kT/vT = (W @ x.T) -> DRAM ============
        with nc.named_scope("passA"), \
             tc.tile_pool(name="xsb", bufs=1) as xpool, \
             tc.tile_pool(name="wA", bufs=6) as wpool, \
             tc.tile_pool(name="psA", bufs=8, space="PSUM") as pspool, \
             tc.tile_pool(name="stA", bufs=4) as stpool:
            xsb = xpool.tile([P, NE, TPC], f32r)
            for e in range(NE):
                nc.sync.dma_start(out=xsb[:, e, :], in_=xT_d[e * P:(e + 1) * P, :])

            wmats = [WqT_d, WkT_d, WvT_d]
            outs = [qT_d, kT_d, vT_d]
            for oi in range(NO):
                wg = []
                for m in range(3):
                    wt = wpool.tile([P, NE, P], f32r, tag="wA")
                    for e in range(NE):
                        nc.sync.dma_start(
                            out=wt[:, e, :],
                            in_=wmats[m][e * P:(e + 1) * P,
                                         oi * P:(oi + 1) * P])
                    wg.append(wt)
                for tcix in range(TPC // TC):
                    for m in range(3):
                        acc = pspool.tile([P, TC], f32, tag="accA")
                        for e in range(NE):
                            nc.tensor.matmul(
                                acc,
                                wg[m][:, e, :],
                                xsb[:, e, tcix * TC:(tcix + 1) * TC],
                                start=(e == 0), stop=(e == NE - 1))
                        st = stpool.tile([P, TC], f32, tag="stA")
                        nc.vector.tensor_copy(st, acc)
                        nc.sync.dma_start(
                            out=outs[m][oi * P:(oi + 1) * P,
                                        tcix * TC:(tcix + 1) * TC],
                            in_=st)

        import os as _os
        if _os.environ.get("KERNEL_PASS_A_ONLY"):
            # debug: skip pass B entirely (output stays unwritten)
            _skip_b = True
        else:
            _skip_b = False
        # ============ PASS B: attention + Wo ============
        NG = TC // 4           # 4-token groups per chunk
        SUB = 64               # tokens per v2 relayout block
        if _skip_b:
            qkvp = None
        if not _skip_b:
         with nc.named_scope("passB"), \
             tc.tile_pool(name="qkv", bufs=1) as qkvp, \
             tc.tile_pool(name="v2p", bufs=1) as v2p, \
             tc.tile_pool(name="bdp", bufs=1) as bdp, \
             tc.tile_pool(name="vgp", bufs=1) as vgp, \
             tc.tile_pool(name="mid", bufs=4) as mid, \
             tc.tile_pool(name="aop", bufs=2) as aop, \
             tc.tile_pool(name="woP", bufs=2) as woP, \
             tc.tile_pool(name="yst", bufs=3) as yst, \
             tc.tile_pool(name="psS", bufs=2, space="PSUM") as psS, \
             tc.tile_pool(name="psM", bufs=4, space="PSUM") as psM, \
             tc.tile_pool(name="psY", bufs=2, space="PSUM") as psY:

            # persistent manually-rotated slots (stable zero padding)
            NBD = 8
            bd_slots = []
            for i in range(NBD):
                t = bdp.tile([P, 64], f32, tag=f"bd{i}")
                nc.vector.memset(t, 0.0)
                bd_slots.append(t)
            NV2 = 2
            v2_slots = []
            for i in range(NV2):
                t = v2p.tile([P, SUB, 32], f32, tag=f"v2_{i}")
                nc.vector.memset(t, 0.0)
                v2_slots.append(t)
            NVG = 8
            vg_slots = []
            for i in range(NVG):
                t = vgp.tile([P, HD + 1], f32, tag=f"vg{i}")
                nc.vector.memset(t[:, HD:HD + 1], 1.0)
                vg_slots.append(t)

            # Wo matmul stream for chunk c-1, interleaved 2 MMs per middle
            # group of chunk c so the PE never idles long enough to cool.
            wo_seq = [(oi, h) for oi in range(NO) for h in range(NH)]

            def wo_step(state, nsteps):
                for _ in range(nsteps):
                    if state is None or state["pos"] >= len(wo_seq):
                        return
                    oi, h = wo_seq[state["pos"]]
                    state["pos"] += 1
                    if h == 0:
                        wo = woP.tile([P, NH, P], f32r, tag="wo", name="wo")
                        nc.sync.dma_start(
                            out=wo,
                            in_=WoT_d[:, oi * P:(oi + 1) * P]
                            .rearrange("(hh p) o -> p hh o", p=P))
                        state["wo"] = wo
                        state["yp"] = psY.tile([P, TC], f32, tag="yps", name="yps")
                    nc.tensor.matmul(
                        state["yp"], state["wo"][:, h, :],
                        state["aoT"][:, h, :],
                        start=(h == 0), stop=(h == NH - 1))
                    if h == NH - 1:
                        ys = yst.tile([P, TC], f32, tag="ys")
                        nc.vector.tensor_copy(ys, state["yp"])
                        nc.sync.dma_start(
                            out=yT_d[oi * P:(oi + 1) * P,
                                     state["t0"]:state["t0"] + TC],
                            in_=ys)

            gi_all = 0
            v2i = 0
            prev = None
            for tcix in range(TPC // TC):
                t0 = tcix * TC
                q_sb = qkvp.tile([P, NH, TC], f32, tag="q")
                k_sb = qkvp.tile([P, NH, TC], f32, tag="k")
                v_sb = qkvp.tile([P, NH, TC], f32, tag="v")
                for g in range(NH):
                    nc.sync.dma_start(out=q_sb[:, g, :],
                                      in_=qT_d[g * P:(g + 1) * P, t0:t0 + TC])
                    nc.sync.dma_start(out=k_sb[:, g, :],
                                      in_=kT_d[g * P:(g + 1) * P, t0:t0 + TC])
                    nc.sync.dma_start(out=v_sb[:, g, :],
                                      in_=vT_d[g * P:(g + 1) * P, t0:t0 + TC])

                aoT = aop.tile([P, NH, TC], f32r, tag="aoT")

                for sub in range(TC // SUB):
                    # relayout v to token-major with padded 32-col slots
                    v2 = v2_slots[v2i % NV2]
                    v2i += 1
                    nc.gpsimd.tensor_copy(
                        v2[:, :, 0:NH],
                        v_sb[:, :, sub * SUB:(sub + 1) * SUB]
                        .rearrange("p g t -> p t g"))

                    for gi4 in range(SUB // 4):
                        tt = sub * SUB + gi4 * 4   # first token in group
                        bd = bd_slots[gi_all % NBD]
                        vg = vg_slots[gi_all % NVG]
                        gi_all += 1

                        # V block transpose: [128, 4*32] -> [4*32, 128]
                        vg_ps = psM.tile([P, P], f32, tag="mps")
                        nc.tensor.transpose(
                            vg_ps,
                            v2[:, gi4 * 4:(gi4 + 1) * 4, :]
                            .rearrange("p t g -> p (t g)"),
                            ident)
                        nc.vector.tensor_copy(vg[:, 0:HD], vg_ps)

                        # scores for 4 tokens -> one psum tile at 32-strips
                        sc_ps = psS.tile([P, NH], f32, tag="scps")
                        for j in range(4):
                            t = tt + j
                            nc.tensor.matmul(
                                sc_ps[32 * j:32 * j + NH, :],
                                k_sb[:, :, t], q_sb[:, :, t],
                                start=True, stop=True,
                                tile_position=(0, 32 * j))
                        # exp for all 4 tokens in one ACT op, then build the
                        # block-diagonal with gpsimd (idle engine) copies
                        es = mid.tile([P, NH], f32, tag="es")
                        nc.scalar.activation(
                            out=es, in_=sc_ps,
                            func=mybir.ActivationFunctionType.Exp,
                            scale=SC)
                        for j in range(4):
                            nc.gpsimd.tensor_copy(
                                bd[32 * j:32 * j + NH, NH * j:NH * (j + 1)],
                                es[32 * j:32 * j + NH, :])

                        # AV: [64,(t,h)] x [128, d+1]
                        av_ps = psM.tile([P, HD + 1], f32, tag="mps")
                        nc.tensor.matmul(av_ps[0:64, :], bd, vg, start=True, stop=True)

                        invz = mid.tile([64, 1], f32, tag="invz")
                        nc.vector.reciprocal(invz, av_ps[0:64, HD:HD + 1])
                        ao = mid.tile([64, HD], f32, tag="ao")
                        nc.vector.tensor_scalar_mul(ao, av_ps[0:64, 0:HD], invz)

                        # transpose back: [64,(t,h) x 128 d] -> [128 d, 64]
                        aoT_ps = psM.tile([P, 64], f32, tag="mps")
                        nc.tensor.transpose(aoT_ps, ao, ident[0:64, 0:64])
                        nc.vector.tensor_copy(
                            aoT[:, :, tt:tt + 4].rearrange("p h t -> p h t"),
                            aoT_ps.rearrange("p (t h) -> p h t", t=4))
                        wo_step(prev, 2)

                # drain any remainder of the previous chunk's Wo stream
                wo_step(prev, len(wo_seq))
                prev = {"pos": 0, "aoT": aoT, "t0": t0, "wo": None, "yp": None}
            wo_step(prev, len(wo_seq))

    nc.compile()
    return nc


def _get_program():
    if "nc" not in _cached:
        _cached["nc"] = _build_program()
    return _cached["nc"]


def kernel(x, Wq, Wk, Wv, Wo):
    from concourse.bass_utils import run_bass_kernel_spmd

    B, S, H = x.shape
    assert (B * S, H) == (NCORES * TPC, E)
    nc = _get_program()

    xf = np.ascontiguousarray(x.reshape(B * S, H))
    WqT = np.ascontiguousarray(Wq.T)
    WkT = np.ascontiguousarray(Wk.T)
    WvT = np.ascontiguousarray(Wv.T)
    WoT = np.ascontiguousarray(Wo.T)

    in_maps = []
    for i in range(NCORES):
        xT = np.ascontiguousarray(xf[i * TPC:(i + 1) * TPC, :].T)
        in_maps.append({"xT": xT, "WqT": WqT, "WkT": WkT,
                        "WvT": WvT, "WoT": WoT})

    import os
    trace = bool(int(os.environ.get("BASS_KERNEL_TRACE", "0")))
    res = run_bass_kernel_spmd(nc, in_maps, core_ids=list(range(NCORES)),
                               trace=trace)
    if trace:
        _cached["last_results"] = res
    parts = [res.results[i]["yT"].T for i in range(NCORES)]
    y = np.concatenate(parts, axis=0).reshape(B, S, H)
    return np.ascontiguousarray(y.astype(np.float32))



# revision 6
# speedup vs baseline: 1.8779x; 1.8779x over previous
"""Trainium2 Bass kernel: per-token multi-head self-attention (fused, bf16).

Computation (per token t):
  q,k,v = x @ W{q,k,v}.T ; scores = (q_t k_t^T)/sqrt(128) over heads [16x16]
  out_t = softmax(scores) @ v_t ; y = out @ Wo.T

Sharding: data-parallel over 16384 tokens -> 8 cores x 2048 tokens.
Per core the 2048 tokens run in 4 chunks of 512, all in one fused pass:

  A(c):   v,q,k for chunk c in [feature, token] layout; weight tiles are
          streamed from DRAM (host pre-tiled, bf16), x chunk resident.
  mid(c): per-token 16x16 head attention. Scores for 128 tokens batch into
          one PSUM bank (4-way tile_position packing), one exp ACT per
          batch, 4 batched DVE copies build the block-diagonal matrix,
          then per 4-token group: V-block PE transpose, one AV matmul with
          a ones-column producing the softmax normalizer, 1/z scale, and a
          PE transpose back. Wo matmuls of chunk c-1 interleave 2-per-group
          as PE filler so the tensor engine never cools.

All matmul operands are bf16 (PSUM accumulation fp32); output y is fp32.
"""
import math
from contextlib import ExitStack

import numpy as np

NCORES = 8
E = 2048          # hidden
NH = 16           # heads
HD = 128          # head dim
TPC = 2048        # tokens per core
TC = 512          # tokens per chunk
P = 128
NE = E // P       # 16 contraction tiles
NO = E // P       # 16 output tiles
CH = TPC // TC    # 4 chunks
NB = TC // P      # 4 score batches (128 tokens) per chunk
NGB = P // 4      # 32 groups of 4 tokens per batch
SUB = 64          # tokens per v relayout block
NSUB = TC // SUB  # 8 per chunk
NV2 = 4           # v2 relayout slots
NVG = 8           # vg slots
SC = 1.0 / math.sqrt(HD)

_cached = {}


def _build_program():
    import concourse.bass as bass
    import concourse.tile as tile
    from concourse import bacc, mybir
    from concourse.masks import make_identity

    f32 = mybir.dt.float32
    bf16 = mybir.dt.bfloat16

    nc = bacc.Bacc("TRN2", target_bir_lowering=False, debug=False)

    x_d = nc.dram_tensor("xt", [CH, P, NE, TC], bf16, kind="ExternalInput").ap()
    w3_d = nc.dram_tensor("w3", [NO, P, 3 * NE, P], bf16, kind="ExternalInput").ap()
    wo_d = nc.dram_tensor("wot", [NO, P, NH, P], bf16, kind="ExternalInput").ap()
    yT_d = nc.dram_tensor("yT", [E, TPC], f32, kind="ExternalOutput").ap()

    with tile.TileContext(nc) as tc, ExitStack() as ctx:
        glob = ctx.enter_context(tc.tile_pool(name="glob", bufs=1))
        xp = ctx.enter_context(tc.tile_pool(name="xp", bufs=2))
        wp = ctx.enter_context(tc.tile_pool(name="wp", bufs=3))
        esp = ctx.enter_context(tc.tile_pool(name="esp", bufs=2))
        aop = ctx.enter_context(tc.tile_pool(name="aop", bufs=2))
        wop = ctx.enter_context(tc.tile_pool(name="wop", bufs=3))
        invp = ctx.enter_context(tc.tile_pool(name="invp", bufs=4))
        aosp = ctx.enter_context(tc.tile_pool(name="aosp", bufs=4))
        ystp = ctx.enter_context(tc.tile_pool(name="ystp", bufs=3))
        psA = ctx.enter_context(tc.tile_pool(name="psA", bufs=2, space="PSUM"))
        psS = ctx.enter_context(tc.tile_pool(name="psS", bufs=1, space="PSUM"))
        psM = ctx.enter_context(tc.tile_pool(name="psM", bufs=3, space="PSUM"))
        psY = ctx.enter_context(tc.tile_pool(name="psY", bufs=2, space="PSUM"))

        ident = glob.tile([P, P], bf16)
        make_identity(nc, ident)

        # persistent chunk-wide activation tiles ([d, head, token] layout)
        qc = glob.tile([P, NH, TC], bf16, tag="qc")
        kc = glob.tile([P, NH, TC], bf16, tag="kc")
        vc = glob.tile([P, NH, TC], bf16, tag="vc")

        # persistent zero-padded slots (padding is memset once, never written)
        v2s = []
        for i in range(NV2):
            t = glob.tile([P, SUB, 32], bf16, tag=f"v2_{i}")
            nc.vector.memset(t, 0.0)
            v2s.append(t)
        bd_slots = []
        for i in range(2):
            t = glob.tile([P, NGB * 64], bf16, tag=f"bd{i}")
            nc.vector.memset(t, 0.0)
            bd_slots.append(t)
        vg_slots = []
        for i in range(NVG):
            t = glob.tile([P, HD + 1], bf16, tag=f"vg{i}")
            nc.vector.memset(t[:, HD:HD + 1], 1.0)
            vg_slots.append(t)

        # ---- weight / x prefetch machinery ----
        w_tiles = []

        def issue_w(oi):
            wt = wp.tile([P, 3 * NE, P], bf16, tag="w", name="wt")
            nc.sync.dma_start(out=wt, in_=w3_d[oi])
            w_tiles.append(wt)

        x_tiles = []

        def issue_x(c):
            xt = xp.tile([P, NE, TC], bf16, tag="xc", name="xt")
            nc.sync.dma_start(out=xt, in_=x_d[c])
            x_tiles.append(xt)

        # ---- Wo interleaved stream over the previous chunk ----
        wo_seq = [(oi, h) for oi in range(NO) for h in range(NH)]

        def wo_prefetch(state):
            oi = state["next_load"]
            if oi < NO:
                wt = wop.tile([P, NH, P], bf16, tag="wo", name="wo")
                nc.sync.dma_start(out=wt, in_=wo_d[oi])
                state["tiles"].append(wt)
                state["next_load"] += 1

        def wo_begin(aoT, t0):
            st = {"pos": 0, "aoT": aoT, "t0": t0, "tiles": [],
                  "next_load": 0, "wo": None, "yp": None}
            wo_prefetch(st)
            wo_prefetch(st)
            return st

        def wo_step(state, nsteps):
            for _ in range(nsteps):
                if state is None or state["pos"] >= len(wo_seq):
                    return
                oi, h = wo_seq[state["pos"]]
                state["pos"] += 1
                if h == 0:
                    state["wo"] = state["tiles"].pop(0)
                    wo_prefetch(state)
                    state["yp"] = psY.tile([P, TC], f32, tag="yp", name="yp")
                nc.tensor.matmul(
                    state["yp"], state["wo"][:, h, :], state["aoT"][:, h, :],
                    start=(h == 0), stop=(h == NH - 1))
                if h == NH - 1:
                    ys = ystp.tile([P, TC], f32, tag="ys")
                    nc.vector.tensor_copy(ys, state["yp"])
                    nc.sync.dma_start(
                        out=yT_d[oi * P:(oi + 1) * P,
                                 state["t0"]:state["t0"] + TC],
                        in_=ys)

        def relayout(s):
            # vc [d, head, tok] -> v2 [d, tok, head(pad 32)] for SUB block s
            v2 = v2s[s % NV2]
            nc.gpsimd.tensor_copy(
                v2[:, :, 0:NH],
                vc[:, :, s * SUB:(s + 1) * SUB].rearrange("p g t -> p t g"))

        dsts = [vc, qc, kc]
        issue_x(0)
        issue_w(0)
        issue_w(1)
        prev = None
        for c in range(CH):
            # ================= A stage: v,q,k for chunk c =================
            with nc.named_scope(f"A{c}"):
                xc = x_tiles.pop(0)
                for oi in range(NO):
                    wt = w_tiles.pop(0)
                    if oi + 2 < NO:
                        issue_w(oi + 2)
                    elif c + 1 < CH:
                        issue_w(oi + 2 - NO)
                    for m in range(3):
                        ps = psA.tile([P, TC], f32, tag="acc")
                        for e in range(NE):
                            nc.tensor.matmul(
                                ps, wt[:, m * NE + e, :], xc[:, e, :],
                                start=(e == 0), stop=(e == NE - 1))
                        nc.vector.tensor_copy(dsts[m][:, oi, :], ps)

            # ================= middle stage (+ Wo of chunk c-1) ============
            with nc.named_scope(f"M{c}"):
                for s in range(NV2):
                    relayout(s)
                if c + 1 < CH:
                    issue_x(c + 1)
                aoT = aop.tile([P, NH, TC], bf16, tag="aoT")
                bd_used = [None] * NB
                for b in range(NB + 1):
                    if b < NB:
                        # scores for 128 tokens into one PSUM bank
                        sc = psS.tile([P, 4 * P], f32, tag="sc")
                        for gi in range(NGB):
                            for j in range(4):
                                t = b * P + gi * 4 + j
                                nc.tensor.matmul(
                                    sc[32 * j:32 * j + NH,
                                       16 * gi:16 * gi + NH],
                                    kc[:, :, t], qc[:, :, t],
                                    start=True, stop=True,
                                    tile_position=(0, 32 * j))
                        es = esp.tile([P, 4 * P], bf16, tag="es")
                        nc.scalar.activation(
                            out=es, in_=sc,
                            func=mybir.ActivationFunctionType.Exp, scale=SC)
                        bd = bd_slots[b % 2]
                        for j in range(4):
                            dst = (bd[32 * j:32 * j + NH, :]
                                   .rearrange("p (gi q) -> p gi q", q=64)
                                   [:, :, 16 * j:16 * j + NH])
                            src = (es[32 * j:32 * j + NH, :]
                                   .rearrange("p (gi h) -> p gi h", h=NH))
                            nc.vector.tensor_copy(dst, src)
                        bd_used[b] = bd
                    if b > 0:
                        bb = b - 1
                        bd = bd_used[bb]
                        for gi in range(NGB):
                            g_c = bb * NGB + gi     # chunk-local group
                            tt = g_c * 4
                            s = g_c // (SUB // 4)   # SUB block
                            gs = g_c % (SUB // 4)
                            v2 = v2s[s % NV2]
                            vg = vg_slots[g_c % NVG]
                            # V block transpose: [d, 4*(tok,g32)] -> [(t,g), d]
                            vg_ps = psM.tile([P, P], bf16, tag="m", name="vg_ps")
                            nc.tensor.transpose(
                                vg_ps,
                                v2[:, gs * 4:(gs + 1) * 4, :]
                                .rearrange("p t g -> p (t g)"),
                                ident)
                            nc.vector.tensor_copy(vg[:, 0:HD], vg_ps)
                            # AV (+ normalizer via ones column)
                            av = psM.tile([P, HD + 4], f32, tag="m", name="av")
                            nc.tensor.matmul(
                                av[0:64, 0:HD + 1],
                                bd[:, 64 * gi:64 * gi + 64], vg,
                                start=True, stop=True)
                            invz = invp.tile([64, 1], f32, tag="invz")
                            nc.vector.reciprocal(invz, av[0:64, HD:HD + 1])
                            ao = aosp.tile([64, HD], bf16, tag="ao")
                            nc.vector.tensor_scalar_mul(
                                ao, av[0:64, 0:HD], invz)
                            # transpose back to [d, (t,h)] and store
                            aoT_ps = psM.tile([P, P], bf16, tag="m", name="aoT_ps")
                            nc.tensor.transpose(
                                aoT_ps[:, 0:64], ao, ident[0:64, 0:64])
                            nc.vector.tensor_copy(
                                aoT[:, :, tt:tt + 4],
                                aoT_ps[:, 0:64]
                                .rearrange("p (t h) -> p h t", t=4))
                            wo_step(prev, 2)
                            if gs == (SUB // 4) - 1 and s + NV2 < NSUB:
                                relayout(s + NV2)
                wo_step(prev, len(wo_seq))
                prev = wo_begin(aoT, c * TC)
        wo_step(prev, len(wo_seq))

    nc.compile()
    return nc


def _get_program():
    if "nc" not in _cached:
        _cached["nc"] = _build_program()
    return _cached["nc"]


def kernel(x, Wq, Wk, Wv, Wo):
    from concourse.bass_utils import run_bass_kernel_spmd
    import ml_dtypes

    bf = ml_dtypes.bfloat16
    B, S, H = x.shape
    assert (B * S, H) == (NCORES * TPC, E)
    nc = _get_program()

    xf = np.asarray(x, dtype=np.float32).reshape(B * S, H)

    def tile_w(WT):
        # WT [E(e-rows), E(f-cols)] -> [NO, P, NE, P] (per-oi contiguous)
        return np.ascontiguousarray(
            WT.reshape(NE, P, NO, P).transpose(2, 1, 0, 3)).astype(bf)

    w3 = np.ascontiguousarray(np.concatenate(
        [tile_w(Wv.T), tile_w(Wq.T), tile_w(Wk.T)], axis=2))
    wo_t = np.ascontiguousarray(
        Wo.T.reshape(NH, P, NO, P).transpose(2, 1, 0, 3)).astype(bf)

    in_maps = []
    for i in range(NCORES):
        xs = xf[i * TPC:(i + 1) * TPC, :].T  # [E, TPC]
        x_t = np.ascontiguousarray(
            xs.reshape(NE, P, CH, TC).transpose(2, 1, 0, 3)).astype(bf)
        in_maps.append({"xt": x_t, "w3": w3, "wot": wo_t})

    import os
    trace = bool(int(os.environ.get("BASS_KERNEL_TRACE", "0")))
    res = run_bass_kernel_spmd(nc, in_maps, core_ids=list(range(NCORES)),
                               trace=trace)
    if trace:
        _cached["last_results"] = res
    parts = [res.results[i]["yT"].T for i in range(NCORES)]
    y = np.concatenate(parts, axis=0).reshape(B, S, H)
    return np.ascontiguousarray(y.astype(np.float32))
